# revision 52
# baseline (speedup 1.0000x reference)
"""Trainium2 Bass kernel for nn_C3S_RegularLoss.

reference:
    xr = x.reshape(B, P, D); xn = xr / ||xr||_2(axis=-1)
    s = mean_b(xn)                     # (P, D)
    corr = s @ s.T                     # (P, P)
    loss = (sum(corr) - 3*trace(corr) + 2P) / 2 * gamma

Reformulated without the corr matrix, with S = sum_b xn (sum, not mean):
    sum(corr)   = ||sum_p S_p||^2 / B^2 = A' / B^2
    trace(corr) = sum_p ||S_p||^2 / B^2 = B2' / B^2
    loss = (A' - 3*B2') * gamma/(2 B^2) + P*gamma

Sharding: data-parallel over the batch dim, 8 cores x 1024 rows.

Design (evolved from trace analysis; ~169us -> ~141us median):
- The x stream (32MB fp32/core, cast to bf16 in-flight by SWDGE) runs
  at the HBM roofline (~99.5us). Every tile's DMA is split per part so
  the ACT sum-of-squares tracks the stream at part granularity instead
  of lagging a whole 12.6us tile; a small leading chunk shortens the
  first-byte latency.
- Per part: ACT square + free accumulator, then one fused
  Abs_reciprocal_sqrt (bf16 out) feeds PE matmuls that accumulate
  S = sum_b r_b*x_b into PSUM (rows 32p), split into column halves A/B
  so the two ship-out readers (ACT copy / DVE copy) run in parallel.
- Last tile: parts 2,3 square in column halves; the ARS bias operand
  fuses the half-sums; part 3's matmul chunks run B-half-first so the
  slower ship path starts earliest.
- Collective: bf16 (4,2048) payload via ONE AllGather (its mesh is just
  the one-shot exchange, ~6us cheaper post-peer than AllReduce); a tiny
  dummy AllReduce fired at t~10us absorbs the one-time ~16us
  collective-stream setup so the real trigger->mesh latency is ~1us.
- Tail (replicated): B2' from two 4-rank (p k) r c loads + a DVE add
  tree + one 128-partition square; A' from a plain (32,2048) load +
  PE ones-matmuls whose 4 chunks land on PSUM partitions 0/32/64/96
  (zeroed at head) + one masked square; X = A'-3B2' accumulates inside
  one PSUM cell via two matmuls; loss = gscale*X + gp in a single ACT
  op with AP scale/bias.
"""

import os
import sys

sys.path.insert(0, "/opt/trn_rl_repo")
os.environ.setdefault("MYCRO_LOCAL_CACHE", "1")

import numpy as np

B, F = 8192, 8192
NPARTS = 4
D = F // NPARTS                 # 2048
NCORES = 8
B_CORE = B // NCORES            # 1024
TILE_P = 128
NTILES = B_CORE // TILE_P       # 8
MM_N = 512                      # moving free dim per matmul
NCHUNK = D // MM_N              # 4
USE_AG = True                   # AllGather + on-core reduce vs AllReduce

_cache = {}


def _build(ncores=NCORES, collective=True):
    import concourse.bass as bass  # noqa: F401
    import concourse.mybir as mybir
    from concourse import bacc, tile
    from concourse.tile import add_dep_helper

    f32 = mybir.dt.float32
    bf16 = mybir.dt.bfloat16
    Act = mybir.ActivationFunctionType
    Alu = mybir.AluOpType
    Ax = mybir.AxisListType

    nc = bacc.Bacc("TRN2", num_devices=ncores, debug=False)
    x_t = nc.dram_tensor("x", [B_CORE, F], f32, kind="ExternalInput")
    g_t = nc.dram_tensor("gamma", [1, 1], f32, kind="ExternalInput")
    out_t = nc.dram_tensor("out", [1, 1], f32, kind="ExternalOutput")

    with tile.TileContext(nc) as tc:
        with tc.tile_pool(name="xp", bufs=7) as xp, \
             tc.tile_pool(name="scratch", bufs=2) as scp, \
             tc.tile_pool(name="small", bufs=3) as stp, \
             tc.tile_pool(name="tail", bufs=1) as tlp, \
             tc.tile_pool(name="ps", bufs=1, space="PSUM") as psp, \
             tc.tile_pool(name="dram", bufs=1, space="DRAM") as dram:

            # PSUM accumulators: part p lives at psum partition 32*p
            # (PE col tile_position constraint); all 8 row-tiles
            # accumulate (start at tile 0, stop at tile 7). Split into
            # column halves A/B so the two ship-out readers (ACT, DVE)
            # are tracked independently and run in parallel.
            S_psA = psp.tile([TILE_P, D // 2], f32, tag="accA")
            S_psB = psp.tile([TILE_P, D // 2], f32, tag="accB")
            if USE_AG:
                # t's 4 column chunks land on PSUM partitions 0/32/64/
                # 96 so its square runs on 4 lanes instead of 1. The
                # tile is zeroed once at the head so the junk rows
                # contribute exact zeros to the masked final matmul.
                t_ps = psp.tile([3 * 32 + 1, MM_N], f32, tag="tps")
                nc.vector.memset(t_ps[:], 0.0)
            X_tile = psp.tile([1, 1], f32, tag="X")
            X_ps = X_tile[:]

            cc_in = dram.tile([NPARTS, D], bf16)
            cc_out = dram.tile([NPARTS, D], bf16)
            cc_outg = dram.tile([ncores * NPARTS, D], bf16)
            cc_w_in = dram.tile([1, 16], f32)
            cc_w_out = dram.tile([1, 16], f32)

            # ---- head: constants + warmup-collective feed ----
            g_sb = tlp.tile([1, 1], f32, tag="g_sb")
            nc.sync.dma_start(g_sb[:], g_t[:])
            # cc_w_in is deliberately never written: the warmup
            # collective reduces garbage (output unused), so its
            # trigger has no data dependency and fires immediately.

            # loss = X * gamma/(2 B^2) + P*gamma, X = A' - 3*B2'
            gscale = tlp.tile([1, 1], f32, tag="gscale")
            nc.vector.tensor_scalar(
                out=gscale[:], in0=g_sb[:],
                scalar1=1.0 / (2.0 * float(B) * float(B)), scalar2=None,
                op0=Alu.mult)
            gp = tlp.tile([1, 1], f32, tag="gp")
            nc.vector.tensor_scalar(
                out=gp[:], in0=g_sb[:], scalar1=float(NPARTS), scalar2=None,
                op0=Alu.mult)

            ones32 = tlp.tile([32, 1], f32, tag="ones32")
            nc.vector.memset(ones32[:], 1.0)
            neg3 = tlp.tile([TILE_P if USE_AG else 32, 1], f32, tag="neg3")
            nc.vector.memset(neg3[:], -3.0)

            ones97 = tlp.tile([3 * 32 + 1, 1], f32, tag="ones97")
            nc.vector.memset(ones97[:], 1.0)

            # ---- main loop over 8 row-tiles ----
            prev_ars = None
            warm_done = False
            for i in range(NTILES):
                last = i == NTILES - 1
                # SWDGE DMA casts fp32 -> bf16 in-flight. First tile:
                # per-part split so the first doorbell rings ~2us
                # earlier (smaller descriptor batch). Last tile: per
                # part so each part's normalize chain starts at its
                # part boundary.
                xt = xp.tile([TILE_P, F], bf16, tag="xt")
                rows = x_t[i * TILE_P:(i + 1) * TILE_P, :]
                # per-part DMAs for EVERY tile: the part-p square only
                # depends on its own quarter, so ACT tracks the stream
                # at part granularity instead of lagging a whole tile.
                # Last tile: additionally split part 3 in column halves.
                for p in range(NPARTS):
                    if last and p >= 2:
                        for h in range(2):
                            c0 = p * D + h * (D // 2)
                            nc.gpsimd.dma_start(xt[:, c0:c0 + D // 2],
                                                rows[:, c0:c0 + D // 2])
                    elif i == 0 and p == 0:
                        # small leading chunk: its SWDGE emission is
                        # short, so the stream's first byte lands ~1.5us
                        # earlier and everything downstream shifts with it
                        nc.gpsimd.dma_start(xt[:, 0:MM_N],
                                            rows[:, 0:MM_N])
                        nc.gpsimd.dma_start(xt[:, MM_N:D],
                                            rows[:, MM_N:D])
                    else:
                        nc.gpsimd.dma_start(xt[:, p * D:(p + 1) * D],
                                            rows[:, p * D:(p + 1) * D])

                if not warm_done:
                    # dummy warmup AllReduce: absorbs the one-time
                    # collective-stream setup (~16us) + mesh crawl
                    # under the DMA stream, so the real AllReduce's
                    # trigger->mesh-begin is ~1us.
                    if collective:
                        nc.gpsimd.collective_compute(
                            "AllReduce", Alu.add,
                            replica_groups=[list(range(ncores))],
                            ins=[cc_w_in.opt()], outs=[cc_w_out.opt()])
                    warm_done = True

                # sum-of-squares per part on ACT (square + free
                # accumulator); r = 1/sqrt(ss) fused on ACT with bf16
                # output. Big elementwise work stays OFF the vector
                # engine mid-stream (DVE SBUF reads lock GpSimd out of
                # the SWDGE descriptor-ring ports).
                ss = stp.tile([TILE_P, NPARTS], f32, tag="ss")
                sqa = scp.tile([TILE_P, D], bf16, tag="sqa")
                r_bf = stp.tile([TILE_P, NPARTS], bf16, tag="r_bf")

                def mms_for_part(p, rbf_ap):
                    # last tile: B-half chunks first so the (slower)
                    # DVE-copy + scalar-DMA ship path starts earliest
                    order = reversed(range(NCHUNK)) if last \
                        else range(NCHUNK)
                    for j in order:
                        S_half = S_psA if j < NCHUNK // 2 else S_psB
                        jj = j % (NCHUNK // 2)
                        nc.tensor.matmul(
                            S_half[32 * p:32 * p + 1,
                                   jj * MM_N:(jj + 1) * MM_N],
                            lhsT=rbf_ap,
                            rhs=xt[:, p * D + j * MM_N:p * D + (j + 1) * MM_N],
                            start=(i == 0),
                            stop=(i == NTILES - 1),
                            tile_position=(0, 32 * p))

                if not last:
                    for p in range(NPARTS):
                        a = nc.scalar.activation(
                            sqa[:], xt[:, p * D:(p + 1) * D], Act.Square,
                            accum_out=ss[:, p:p + 1])
                        if p == 0 and prev_ars is not None:
                            # pin ACT order: ars(i-1) must precede
                            # squares(i), else the scheduler makes
                            # r(i-1) wait on DMA(i)
                            add_dep_helper(
                                a.ins, prev_ars.ins, sync=False,
                                reason="ars(i-1) before squares(i)")
                    prev_ars = nc.scalar.activation(
                        r_bf[:], ss[:], Act.Abs_reciprocal_sqrt)
                    for p in range(NPARTS):
                        mms_for_part(p, r_bf[:, p:p + 1])
                else:
                    # last tile: per-part chains all on ACT. Part 3
                    # squares in column halves with separate accums;
                    # the ARS's bias operand fuses the half-sum
                    # (ars = 1/sqrt(1.0*ss_b + ss_a)) with no DVE hop.
                    ssh = stp.tile([TILE_P, 4], f32, tag="ssh")
                    chain = []
                    for p in (0, 1):
                        a = nc.scalar.activation(
                            sqa[:], xt[:, p * D:(p + 1) * D], Act.Square,
                            accum_out=ss[:, p:p + 1])
                        if p == 0 and prev_ars is not None:
                            add_dep_helper(a.ins, prev_ars.ins, sync=False,
                                           reason="ars(i-1) first")
                        chain.append(a)
                        chain.append(nc.scalar.activation(
                            r_bf[:, p:p + 1], ss[:, p:p + 1],
                            Act.Abs_reciprocal_sqrt))
                        mms_for_part(p, r_bf[:, p:p + 1])
                    p = 2
                    for h in range(2):
                        c0 = p * D + h * (D // 2)
                        chain.append(nc.scalar.activation(
                            sqa[:, :D // 2], xt[:, c0:c0 + D // 2],
                            Act.Square, accum_out=ssh[:, h:h + 1]))
                    chain.append(nc.scalar.activation(
                        r_bf[:, p:p + 1], ssh[:, 1:2],
                        Act.Abs_reciprocal_sqrt, bias=ssh[:, 0:1]))
                    mms_for_part(p, r_bf[:, p:p + 1])
                    # part 3: first half on the idle DVE (mult+reduce),
                    # second half on ACT, so ACT starts the final
                    # square exactly when the last bytes land; the ARS
                    # bias fuses the two engines' accumulators.
                    p = 3
                    sqv = scp.tile([TILE_P, D // 2], bf16, tag="sqv")
                    nc.vector.tensor_mul(sqv[:], xt[:, p * D:p * D + D // 2],
                                         xt[:, p * D:p * D + D // 2])
                    nc.vector.tensor_reduce(out=ssh[:, 2:3], in_=sqv[:],
                                            axis=Ax.X, op=Alu.add)
                    chain.append(nc.scalar.activation(
                        sqa[:, :D // 2], xt[:, p * D + D // 2:(p + 1) * D],
                        Act.Square, accum_out=ssh[:, 3:4]))
                    chain.append(nc.scalar.activation(
                        r_bf[:, p:p + 1], ssh[:, 3:4],
                        Act.Abs_reciprocal_sqrt, bias=ssh[:, 2:3]))
                    mms_for_part(p, r_bf[:, p:p + 1])
                    for a, b in zip(chain, chain[1:]):
                        add_dep_helper(b.ins, a.ins, sync=False,
                                       reason="ACT order last tile")

            # ---- ship the 4 used PSUM rows out as bf16 ----
            # Full-width copies (junk rows besides 0/32/64/96 are
            # harmless) split into column halves on ACT and DVE; two
            # separate destination tiles so the engines are not
            # serialized by tile-granular write tracking. Partition
            # stride lives in the DMA access patterns.
            sA = tlp.tile([TILE_P, D // 2], bf16, tag="sA")
            sV = tlp.tile([TILE_P, D // 2], bf16, tag="sV")
            nc.scalar.copy(sA[:], S_psA[:])
            nc.vector.tensor_copy(sV[:], S_psB[:])
            # both halves on the sync HWDGE ring: the scalar ring's
            # DIRECT2D is consistently ~0.5us slower
            nc.sync.dma_start(cc_in[:, :D // 2], sA[0:3 * 32 + 1:32, :])
            nc.sync.dma_start(cc_in[:, D // 2:], sV[0:3 * 32 + 1:32, :])

            if USE_AG:
                # ---- AllGather + on-core reduce ----
                # The AllReduce mesh spends ~8us in post-gather reduce/
                # redistribute phases; AllGather is just the one-shot
                # exchange, and the 8-rank sum is a DVE add tree that
                # pipelines with the per-rank loads.
                if collective:
                    nc.gpsimd.collective_compute(
                        "AllGather", Alu.bypass,
                        replica_groups=[list(range(ncores))],
                        ins=[cc_in.opt()], outs=[cc_outg.opt()])
                else:
                    for r in range(ncores):
                        nc.sync.dma_start(
                            cc_outg[r * NPARTS:(r + 1) * NPARTS, :],
                            cc_in[:])
                # A-side: t = sum over ALL 32 gathered rows (ranks x
                # parts) via PE ones-matmuls, chunk j landing on PSUM
                # partition 32j. G32 is loaded as two half-tiles so the
                # first two matmuls start after the first half lands.
                ones32b = tlp.tile([32, 1], bf16, tag="ones32b")
                nc.vector.memset(ones32b[:], 1.0)
                G32h = []
                for h in range(2):
                    t = tlp.tile([4 * ncores, D // 2], bf16, tag=f"G32{h}")
                    nc.sync.dma_start(
                        t[:], cc_outg[:, h * (D // 2):(h + 1) * (D // 2)])
                    G32h.append(t)
                for j in range(NCHUNK):
                    nc.tensor.matmul(
                        t_ps[32 * j:32 * j + 1, :],
                        lhsT=ones32b[:],
                        rhs=G32h[j // 2][:, (j % 2) * MM_N:
                                         (j % 2 + 1) * MM_N],
                        start=True, stop=True,
                        tile_position=(0, 32 * j))
                sqA = tlp.tile([3 * 32 + 1, MM_N], bf16, tag="sqA")
                ssA = tlp.tile([3 * 32 + 1, 1], f32, tag="ssA")
                nc.scalar.activation(sqA[:], t_ps[:], Act.Square,
                                     accum_out=ssA[:])

                # B2-side: ONE 8-rank load in the (p k) r c layout
                # (ranks side by side in columns); the 8-rank sum is 3
                # chained column adds on DVE (in-order, no cross-engine
                # semaphores).
                L8 = tlp.tile([TILE_P, 8 * 64], bf16, tag="L8")
                nc.scalar.dma_start(
                    L8[:].rearrange("q (r c) -> q r c", r=8),
                    cc_outg[:]
                    .rearrange("(r p) (k c) -> (p k) r c", p=4, k=32))
                a4 = tlp.tile([TILE_P, 4 * 64], bf16, tag="a4")
                nc.vector.tensor_add(a4[:], L8[:, 0:256], L8[:, 256:512])
                a2 = tlp.tile([TILE_P, 2 * 64], bf16, tag="a2")
                nc.vector.tensor_add(a2[:], a4[:, 0:128], a4[:, 128:256])
                T128s = tlp.tile([TILE_P, 64], bf16, tag="T128s")
                nc.vector.tensor_add(T128s[:], a2[:, 0:64], a2[:, 64:128])

                sqB = tlp.tile([TILE_P, 64], bf16, tag="sqB")
                ssB = tlp.tile([TILE_P, 1], f32, tag="ssB")
                nc.scalar.activation(sqB[:], T128s[:], Act.Square,
                                     accum_out=ssB[:])
            else:
                if collective:
                    nc.gpsimd.collective_compute(
                        "AllReduce", Alu.add,
                        replica_groups=[list(range(ncores))],
                        ins=[cc_in.opt()], outs=[cc_out.opt()])
                else:
                    nc.sync.dma_start(cc_out[:], cc_in[:])

                # ---- replicated tail on a (32,256) view ----
                # Tc[k, p*64+c] = S[p, k*64+c]: part p is a 64-wide
                # column block, so the cross-part sum is column-wise
                # DVE adds and every reduction uses 32 partitions.
                Tc = tlp.tile([32, 4 * 64], bf16, tag="Tc")
                for p in range(NPARTS):
                    eng = nc.sync if p % 2 == 0 else nc.scalar
                    eng.dma_start(
                        Tc[:, p * 64:(p + 1) * 64],
                        cc_out[p:p + 1, :]
                        .rearrange("o (k c) -> (o k) c", k=32))

                sqB = tlp.tile([32, 4 * 64], bf16, tag="sqB")
                ssB = tlp.tile([32, 1], f32, tag="ssB")
                nc.scalar.activation(sqB[:], Tc[:], Act.Square,
                                     accum_out=ssB[:])
                u32 = tlp.tile([32, 64], bf16, tag="u32")
                v32 = tlp.tile([32, 64], bf16, tag="v32")
                t32 = tlp.tile([32, 64], bf16, tag="t32")
                nc.vector.tensor_add(u32[:], Tc[:, 0:64], Tc[:, 64:128])
                nc.vector.tensor_add(v32[:], Tc[:, 128:192],
                                     Tc[:, 192:256])
                nc.vector.tensor_add(t32[:], u32[:], v32[:])
                sqA = tlp.tile([32, 64], bf16, tag="sqA")
                ssA = tlp.tile([32, 1], f32, tag="ssA")
                nc.scalar.activation(sqA[:], t32[:], Act.Square,
                                     accum_out=ssA[:])

            # X = A' - 3*B2' accumulated inside one PSUM cell
            nc.tensor.matmul(X_ps, lhsT=ones97[:] if USE_AG
                             else ones32[:], rhs=ssA[:],
                             start=True, stop=False)
            nc.tensor.matmul(X_ps, lhsT=neg3[:], rhs=ssB[:],
                             start=False, stop=True)
            # loss = gscale * X + gp, single ACT op
            loss = tlp.tile([1, 1], f32, tag="loss")
            nc.scalar.activation(loss[:], X_ps, Act.Identity,
                                 bias=gp[0:1, 0:1], scale=gscale[0:1, 0:1])
            nc.sync.dma_start(out_t[:], loss[:])

    nc.compile()
    return nc


def _get_nc():
    if "nc" not in _cache:
        _cache["nc"] = _build()
    return _cache["nc"]


def kernel(x, gamma, **run_kwargs):
    from concourse import bass_utils

    x = np.ascontiguousarray(np.asarray(x, dtype=np.float32))
    gamma = np.asarray(gamma, dtype=np.float32).reshape(1, 1)
    assert x.shape == (B, F), x.shape

    nc = _get_nc()
    in_maps = [
        {"x": x[c * B_CORE:(c + 1) * B_CORE], "gamma": gamma}
        for c in range(NCORES)
    ]
    res = bass_utils.run_bass_kernel_spmd(
        nc, in_maps, core_ids=list(range(NCORES)), **run_kwargs)
    out = np.asarray(res.results[0]["out"], dtype=np.float32).reshape(1)
    if run_kwargs.get("trace"):
        _cache["last_results"] = res
    return out


# revision 54
# speedup vs baseline: 1.0315x; 1.0315x over previous
"""Trainium2 Bass kernel for nn_C3S_RegularLoss.

reference:
    xr = x.reshape(B, P, D); xn = xr / ||xr||_2(axis=-1)
    s = mean_b(xn)                     # (P, D)
    corr = s @ s.T                     # (P, P)
    loss = (sum(corr) - 3*trace(corr) + 2P) / 2 * gamma

Reformulated without the corr matrix, with S = sum_b xn (sum, not mean):
    sum(corr)   = ||sum_p S_p||^2 / B^2 = A' / B^2
    trace(corr) = sum_p ||S_p||^2 / B^2 = B2' / B^2
    loss = (A' - 3*B2') * gamma/(2 B^2) + P*gamma

Sharding: data-parallel over the batch dim, 8 cores x 1024 rows.

Design (evolved from trace analysis; ~169us -> ~141us median):
- The x stream (32MB fp32/core, cast to bf16 in-flight by SWDGE) runs
  at the HBM roofline (~99.5us). Every tile's DMA is split per part so
  the ACT sum-of-squares tracks the stream at part granularity instead
  of lagging a whole 12.6us tile; a small leading chunk shortens the
  first-byte latency.
- Per part: ACT square + free accumulator, then one fused
  Abs_reciprocal_sqrt (bf16 out) feeds PE matmuls that accumulate
  S = sum_b r_b*x_b into PSUM (rows 32p), split into column halves A/B
  so the two ship-out readers (ACT copy / DVE copy) run in parallel.
- Last tile: parts 2,3 square in column halves; the ARS bias operand
  fuses the half-sums; part 3's matmul chunks run B-half-first so the
  slower ship path starts earliest.
- Collective: bf16 (4,2048) payload via ONE AllGather (its mesh is just
  the one-shot exchange, ~6us cheaper post-peer than AllReduce); a tiny
  dummy AllReduce fired at t~10us absorbs the one-time ~16us
  collective-stream setup so the real trigger->mesh latency is ~1us.
- Tail (replicated): B2' from two 4-rank (p k) r c loads + a DVE add
  tree + one 128-partition square; A' from a plain (32,2048) load +
  PE ones-matmuls whose 4 chunks land on PSUM partitions 0/32/64/96
  (zeroed at head) + one masked square; X = A'-3B2' accumulates inside
  one PSUM cell via two matmuls; loss = gscale*X + gp in a single ACT
  op with AP scale/bias.
"""

import os
import sys

sys.path.insert(0, "/opt/trn_rl_repo")
os.environ.setdefault("MYCRO_LOCAL_CACHE", "1")

import numpy as np

B, F = 8192, 8192
NPARTS = 4
D = F // NPARTS                 # 2048
NCORES = 8
B_CORE = B // NCORES            # 1024
TILE_P = 128
NTILES = B_CORE // TILE_P       # 8
MM_N = 512                      # moving free dim per matmul
NCHUNK = D // MM_N              # 4
USE_AG = True                   # AllGather + on-core reduce vs AllReduce

_cache = {}


def _build(ncores=NCORES, collective=True):
    import concourse.bass as bass  # noqa: F401
    import concourse.mybir as mybir
    from concourse import bacc, tile
    from concourse.tile import add_dep_helper

    f32 = mybir.dt.float32
    bf16 = mybir.dt.bfloat16
    Act = mybir.ActivationFunctionType
    Alu = mybir.AluOpType
    Ax = mybir.AxisListType

    nc = bacc.Bacc("TRN2", num_devices=ncores, debug=False)
    x_t = nc.dram_tensor("x", [B_CORE, F], f32, kind="ExternalInput")
    g_t = nc.dram_tensor("gamma", [1, 1], f32, kind="ExternalInput")
    out_t = nc.dram_tensor("out", [1, 1], f32, kind="ExternalOutput")

    with tile.TileContext(nc) as tc:
        with tc.tile_pool(name="xp", bufs=7) as xp, \
             tc.tile_pool(name="scratch", bufs=2) as scp, \
             tc.tile_pool(name="small", bufs=3) as stp, \
             tc.tile_pool(name="tail", bufs=1) as tlp, \
             tc.tile_pool(name="ps", bufs=1, space="PSUM") as psp, \
             tc.tile_pool(name="dram", bufs=1, space="DRAM") as dram:

            # PSUM accumulators: part p lives at psum partition 32*p
            # (PE col tile_position constraint); all 8 row-tiles
            # accumulate (start at tile 0, stop at tile 7). Split into
            # column halves A/B so the two ship-out readers (ACT, DVE)
            # are tracked independently and run in parallel.
            S_psA = psp.tile([TILE_P, D // 2], f32, tag="accA")
            S_psB = psp.tile([TILE_P, D // 2], f32, tag="accB")
            if USE_AG:
                # t's 4 column chunks land on PSUM partitions 0/32/64/
                # 96 so its square runs on 4 lanes instead of 1. The
                # tile is zeroed once at the head so the junk rows
                # contribute exact zeros to the masked final matmul.
                t_ps = psp.tile([3 * 32 + 1, MM_N], f32, tag="tps")
                nc.vector.memset(t_ps[:], 0.0)
            X_tile = psp.tile([1, 1], f32, tag="X")
            X_ps = X_tile[:]

            cc_in = dram.tile([NPARTS, D], bf16)
            cc_out = dram.tile([NPARTS, D], bf16)
            cc_outg = dram.tile([ncores * NPARTS, D], bf16)
            cc_w_in = dram.tile([1, 16], f32)
            cc_w_out = dram.tile([1, 16], f32)

            # ---- head: constants + warmup-collective feed ----
            g_sb = tlp.tile([1, 1], f32, tag="g_sb")
            nc.sync.dma_start(g_sb[:], g_t[:])
            # cc_w_in is deliberately never written: the warmup
            # collective reduces garbage (output unused), so its
            # trigger has no data dependency and fires immediately.

            # loss = X * gamma/(2 B^2) + P*gamma, X = A' - 3*B2'
            gscale = tlp.tile([1, 1], f32, tag="gscale")
            nc.vector.tensor_scalar(
                out=gscale[:], in0=g_sb[:],
                scalar1=1.0 / (2.0 * float(B) * float(B)), scalar2=None,
                op0=Alu.mult)
            gp = tlp.tile([1, 1], f32, tag="gp")
            nc.vector.tensor_scalar(
                out=gp[:], in0=g_sb[:], scalar1=float(NPARTS), scalar2=None,
                op0=Alu.mult)

            ones32 = tlp.tile([32, 1], f32, tag="ones32")
            nc.vector.memset(ones32[:], 1.0)
            neg3 = tlp.tile([TILE_P if USE_AG else 32, 1], f32, tag="neg3")
            nc.vector.memset(neg3[:], -3.0)

            ones97 = tlp.tile([3 * 32 + 1, 1], f32, tag="ones97")
            nc.vector.memset(ones97[:], 1.0)

            # ---- main loop over 8 row-tiles ----
            prev_ars = None
            warm_done = False
            for i in range(NTILES):
                last = i == NTILES - 1
                # SWDGE DMA casts fp32 -> bf16 in-flight. First tile:
                # per-part split so the first doorbell rings ~2us
                # earlier (smaller descriptor batch). Last tile: per
                # part so each part's normalize chain starts at its
                # part boundary.
                xt = xp.tile([TILE_P, F], bf16, tag="xt")
                rows = x_t[i * TILE_P:(i + 1) * TILE_P, :]
                # per-part DMAs for EVERY tile: the part-p square only
                # depends on its own quarter, so ACT tracks the stream
                # at part granularity instead of lagging a whole tile.
                # Last tile: additionally split part 3 in column halves.
                if last:
                    # part 3 is loaded as fp8_e4m3 (in-flight cast):
                    # its four matmuls sit on the collective-trigger
                    # path and fp8 runs the PE at 2x column rate. The
                    # loss has ~1e3x precision headroom.
                    x8 = tlp.tile([TILE_P, D], mybir.dt.float8e4,
                                  tag="x8")
                for p in range(NPARTS):
                    if last and p == 3:
                        for h in range(2):
                            nc.gpsimd.dma_start(
                                x8[:, h * (D // 2):(h + 1) * (D // 2)],
                                rows[:, p * D + h * (D // 2):
                                     p * D + (h + 1) * (D // 2)])
                    elif last and p == 2:
                        for h in range(2):
                            c0 = p * D + h * (D // 2)
                            nc.gpsimd.dma_start(xt[:, c0:c0 + D // 2],
                                                rows[:, c0:c0 + D // 2])
                    elif i == 0 and p == 0:
                        # small leading chunk: its SWDGE emission is
                        # short, so the stream's first byte lands ~1.5us
                        # earlier and everything downstream shifts with it
                        nc.gpsimd.dma_start(xt[:, 0:MM_N],
                                            rows[:, 0:MM_N])
                        nc.gpsimd.dma_start(xt[:, MM_N:D],
                                            rows[:, MM_N:D])
                    else:
                        nc.gpsimd.dma_start(xt[:, p * D:(p + 1) * D],
                                            rows[:, p * D:(p + 1) * D])

                if not warm_done:
                    # dummy warmup AllReduce: absorbs the one-time
                    # collective-stream setup (~16us) + mesh crawl
                    # under the DMA stream, so the real AllReduce's
                    # trigger->mesh-begin is ~1us.
                    if collective:
                        nc.gpsimd.collective_compute(
                            "AllReduce", Alu.add,
                            replica_groups=[list(range(ncores))],
                            ins=[cc_w_in.opt()], outs=[cc_w_out.opt()])
                    warm_done = True

                # sum-of-squares per part on ACT (square + free
                # accumulator); r = 1/sqrt(ss) fused on ACT with bf16
                # output. Big elementwise work stays OFF the vector
                # engine mid-stream (DVE SBUF reads lock GpSimd out of
                # the SWDGE descriptor-ring ports).
                ss = stp.tile([TILE_P, NPARTS], f32, tag="ss")
                sqa = scp.tile([TILE_P, D], bf16, tag="sqa")
                r_bf = stp.tile([TILE_P, NPARTS], bf16, tag="r_bf")

                def mms_for_part(p, rbf_ap):
                    # last tile: B-half chunks first so the (slower)
                    # DVE-copy + scalar-DMA ship path starts earliest
                    order = reversed(range(NCHUNK)) if last \
                        else range(NCHUNK)
                    for j in order:
                        S_half = S_psA if j < NCHUNK // 2 else S_psB
                        jj = j % (NCHUNK // 2)
                        nc.tensor.matmul(
                            S_half[32 * p:32 * p + 1,
                                   jj * MM_N:(jj + 1) * MM_N],
                            lhsT=rbf_ap,
                            rhs=xt[:, p * D + j * MM_N:p * D + (j + 1) * MM_N],
                            start=(i == 0),
                            stop=(i == NTILES - 1),
                            tile_position=(0, 32 * p))

                if not last:
                    for p in range(NPARTS):
                        a = nc.scalar.activation(
                            sqa[:], xt[:, p * D:(p + 1) * D], Act.Square,
                            accum_out=ss[:, p:p + 1])
                        if p == 0 and prev_ars is not None:
                            # pin ACT order: ars(i-1) must precede
                            # squares(i), else the scheduler makes
                            # r(i-1) wait on DMA(i)
                            add_dep_helper(
                                a.ins, prev_ars.ins, sync=False,
                                reason="ars(i-1) before squares(i)")
                    prev_ars = nc.scalar.activation(
                        r_bf[:], ss[:], Act.Abs_reciprocal_sqrt)
                    for p in range(NPARTS):
                        mms_for_part(p, r_bf[:, p:p + 1])
                else:
                    # last tile: per-part chains all on ACT. Part 3
                    # squares in column halves with separate accums;
                    # the ARS's bias operand fuses the half-sum
                    # (ars = 1/sqrt(1.0*ss_b + ss_a)) with no DVE hop.
                    ssh = stp.tile([TILE_P, 4], f32, tag="ssh")
                    chain = []
                    for p in (0, 1):
                        a = nc.scalar.activation(
                            sqa[:], xt[:, p * D:(p + 1) * D], Act.Square,
                            accum_out=ss[:, p:p + 1])
                        if p == 0 and prev_ars is not None:
                            add_dep_helper(a.ins, prev_ars.ins, sync=False,
                                           reason="ars(i-1) first")
                        chain.append(a)
                        chain.append(nc.scalar.activation(
                            r_bf[:, p:p + 1], ss[:, p:p + 1],
                            Act.Abs_reciprocal_sqrt))
                        mms_for_part(p, r_bf[:, p:p + 1])
                    p = 2
                    for h in range(2):
                        c0 = p * D + h * (D // 2)
                        chain.append(nc.scalar.activation(
                            sqa[:, :D // 2], xt[:, c0:c0 + D // 2],
                            Act.Square, accum_out=ssh[:, h:h + 1]))
                    chain.append(nc.scalar.activation(
                        r_bf[:, p:p + 1], ssh[:, 1:2],
                        Act.Abs_reciprocal_sqrt, bias=ssh[:, 0:1]))
                    mms_for_part(p, r_bf[:, p:p + 1])
                    # part 3 (fp8): first half on the idle DVE
                    # (mult+reduce), second half on ACT, so ACT starts
                    # the final square exactly when the last bytes
                    # land; the ARS bias fuses the two engines'
                    # accumulators and emits an fp8 reciprocal for the
                    # 2x-rate PE matmuls.
                    sqv = scp.tile([TILE_P, D // 2], bf16, tag="sqv")
                    nc.vector.tensor_mul(sqv[:], x8[:, :D // 2],
                                         x8[:, :D // 2])
                    nc.vector.tensor_reduce(out=ssh[:, 2:3], in_=sqv[:],
                                            axis=Ax.X, op=Alu.add)
                    chain.append(nc.scalar.activation(
                        sqa[:, :D // 2], x8[:, D // 2:],
                        Act.Square, accum_out=ssh[:, 3:4]))
                    r8 = stp.tile([TILE_P, 1], mybir.dt.float8e4,
                                  tag="r8")
                    chain.append(nc.scalar.activation(
                        r8[:], ssh[:, 3:4],
                        Act.Abs_reciprocal_sqrt, bias=ssh[:, 2:3]))
                    for j in reversed(range(NCHUNK)):
                        S_half = S_psA if j < NCHUNK // 2 else S_psB
                        jj = j % (NCHUNK // 2)
                        nc.tensor.matmul(
                            S_half[96:97, jj * MM_N:(jj + 1) * MM_N],
                            lhsT=r8[:],
                            rhs=x8[:, j * MM_N:(j + 1) * MM_N],
                            start=False, stop=True,
                            tile_position=(0, 96))
                    for a, b in zip(chain, chain[1:]):
                        add_dep_helper(b.ins, a.ins, sync=False,
                                       reason="ACT order last tile")

            # ---- ship the 4 used PSUM rows out as bf16 ----
            # Full-width copies (junk rows besides 0/32/64/96 are
            # harmless) split into column halves on ACT and DVE; two
            # separate destination tiles so the engines are not
            # serialized by tile-granular write tracking. Partition
            # stride lives in the DMA access patterns.
            sA = tlp.tile([TILE_P, D // 2], bf16, tag="sA")
            sV = tlp.tile([TILE_P, D // 2], bf16, tag="sV")
            nc.scalar.copy(sA[:], S_psA[:])
            nc.vector.tensor_copy(sV[:], S_psB[:])
            # both halves on the sync HWDGE ring: the scalar ring's
            # DIRECT2D is consistently ~0.5us slower
            nc.sync.dma_start(cc_in[:, :D // 2], sA[0:3 * 32 + 1:32, :])
            nc.sync.dma_start(cc_in[:, D // 2:], sV[0:3 * 32 + 1:32, :])

            if USE_AG:
                # ---- AllGather + on-core reduce ----
                # The AllReduce mesh spends ~8us in post-gather reduce/
                # redistribute phases; AllGather is just the one-shot
                # exchange, and the 8-rank sum is a DVE add tree that
                # pipelines with the per-rank loads.
                if collective:
                    nc.gpsimd.collective_compute(
                        "AllGather", Alu.bypass,
                        replica_groups=[list(range(ncores))],
                        ins=[cc_in.opt()], outs=[cc_outg.opt()])
                else:
                    for r in range(ncores):
                        nc.sync.dma_start(
                            cc_outg[r * NPARTS:(r + 1) * NPARTS, :],
                            cc_in[:])
                # A-side: t = sum over ALL 32 gathered rows (ranks x
                # parts) via PE ones-matmuls, chunk j landing on PSUM
                # partition 32j. G32 is loaded as two half-tiles so the
                # first two matmuls start after the first half lands.
                ones32b = tlp.tile([32, 1], bf16, tag="ones32b")
                nc.vector.memset(ones32b[:], 1.0)
                G32h = []
                for h in range(2):
                    t = tlp.tile([4 * ncores, D // 2], bf16, tag=f"G32{h}")
                    nc.sync.dma_start(
                        t[:], cc_outg[:, h * (D // 2):(h + 1) * (D // 2)])
                    G32h.append(t)
                for j in range(NCHUNK):
                    nc.tensor.matmul(
                        t_ps[32 * j:32 * j + 1, :],
                        lhsT=ones32b[:],
                        rhs=G32h[j // 2][:, (j % 2) * MM_N:
                                         (j % 2 + 1) * MM_N],
                        start=True, stop=True,
                        tile_position=(0, 32 * j))
                sqA = tlp.tile([3 * 32 + 1, MM_N], bf16, tag="sqA")
                ssA = tlp.tile([3 * 32 + 1, 1], f32, tag="ssA")
                nc.scalar.activation(sqA[:], t_ps[:], Act.Square,
                                     accum_out=ssA[:])

                # B2-side: ONE 8-rank load in the (p k) r c layout
                # (ranks side by side in columns); the 8-rank sum is 3
                # chained column adds on DVE (in-order, no cross-engine
                # semaphores).
                L8 = tlp.tile([TILE_P, 8 * 64], bf16, tag="L8")
                nc.scalar.dma_start(
                    L8[:].rearrange("q (r c) -> q r c", r=8),
                    cc_outg[:]
                    .rearrange("(r p) (k c) -> (p k) r c", p=4, k=32))
                a4 = tlp.tile([TILE_P, 4 * 64], bf16, tag="a4")
                nc.vector.tensor_add(a4[:], L8[:, 0:256], L8[:, 256:512])
                a2 = tlp.tile([TILE_P, 2 * 64], bf16, tag="a2")
                nc.vector.tensor_add(a2[:], a4[:, 0:128], a4[:, 128:256])
                T128s = tlp.tile([TILE_P, 64], bf16, tag="T128s")
                nc.vector.tensor_add(T128s[:], a2[:, 0:64], a2[:, 64:128])

                sqB = tlp.tile([TILE_P, 64], bf16, tag="sqB")
                ssB = tlp.tile([TILE_P, 1], f32, tag="ssB")
                nc.scalar.activation(sqB[:], T128s[:], Act.Square,
                                     accum_out=ssB[:])
            else:
                if collective:
                    nc.gpsimd.collective_compute(
                        "AllReduce", Alu.add,
                        replica_groups=[list(range(ncores))],
                        ins=[cc_in.opt()], outs=[cc_out.opt()])
                else:
                    nc.sync.dma_start(cc_out[:], cc_in[:])

                # ---- replicated tail on a (32,256) view ----
                # Tc[k, p*64+c] = S[p, k*64+c]: part p is a 64-wide
                # column block, so the cross-part sum is column-wise
                # DVE adds and every reduction uses 32 partitions.
                Tc = tlp.tile([32, 4 * 64], bf16, tag="Tc")
                for p in range(NPARTS):
                    eng = nc.sync if p % 2 == 0 else nc.scalar
                    eng.dma_start(
                        Tc[:, p * 64:(p + 1) * 64],
                        cc_out[p:p + 1, :]
                        .rearrange("o (k c) -> (o k) c", k=32))

                sqB = tlp.tile([32, 4 * 64], bf16, tag="sqB")
                ssB = tlp.tile([32, 1], f32, tag="ssB")
                nc.scalar.activation(sqB[:], Tc[:], Act.Square,
                                     accum_out=ssB[:])
                u32 = tlp.tile([32, 64], bf16, tag="u32")
                v32 = tlp.tile([32, 64], bf16, tag="v32")
                t32 = tlp.tile([32, 64], bf16, tag="t32")
                nc.vector.tensor_add(u32[:], Tc[:, 0:64], Tc[:, 64:128])
                nc.vector.tensor_add(v32[:], Tc[:, 128:192],
                                     Tc[:, 192:256])
                nc.vector.tensor_add(t32[:], u32[:], v32[:])
                sqA = tlp.tile([32, 64], bf16, tag="sqA")
                ssA = tlp.tile([32, 1], f32, tag="ssA")
                nc.scalar.activation(sqA[:], t32[:], Act.Square,
                                     accum_out=ssA[:])

            # X = A' - 3*B2' accumulated inside one PSUM cell
            nc.tensor.matmul(X_ps, lhsT=ones97[:] if USE_AG
                             else ones32[:], rhs=ssA[:],
                             start=True, stop=False)
            nc.tensor.matmul(X_ps, lhsT=neg3[:], rhs=ssB[:],
                             start=False, stop=True)
            # loss = gscale * X + gp, single ACT op
            loss = tlp.tile([1, 1], f32, tag="loss")
            nc.scalar.activation(loss[:], X_ps, Act.Identity,
                                 bias=gp[0:1, 0:1], scale=gscale[0:1, 0:1])
            nc.sync.dma_start(out_t[:], loss[:])

    nc.compile()
    return nc


def _get_nc():
    if "nc" not in _cache:
        _cache["nc"] = _build()
    return _cache["nc"]


def kernel(x, gamma, **run_kwargs):
    from concourse import bass_utils

    x = np.ascontiguousarray(np.asarray(x, dtype=np.float32))
    gamma = np.asarray(gamma, dtype=np.float32).reshape(1, 1)
    assert x.shape == (B, F), x.shape

    nc = _get_nc()
    in_maps = [
        {"x": x[c * B_CORE:(c + 1) * B_CORE], "gamma": gamma}
        for c in range(NCORES)
    ]
    res = bass_utils.run_bass_kernel_spmd(
        nc, in_maps, core_ids=list(range(NCORES)), **run_kwargs)
    out = np.asarray(res.results[0]["out"], dtype=np.float32).reshape(1)
    if run_kwargs.get("trace"):
        _cache["last_results"] = res
    return out
